# revision 8
# baseline (speedup 1.0000x reference)
"""ArcFace (AngularPenaltySMLoss) forward on 8 Trainium2 NeuronCores.

reference:
    target    = wf[i, labels[i]]                         # [B] gather
    numerator = S * cos(arccos(clip(target)) + M)
    excl_sum  = sum_j exp(S*wf[i,j]) - exp(S*target)
    L_i       = numerator - log(exp(numerator) + excl_sum)
    out       = -mean(L)

Strategy (data parallel over the batch):
  - 8 cores x 512 rows each. Each core streams its 512x50257 f32 shard of wf
    (103 MB) through SBUF; ScalarE computes exp(S*x) with the activation
    instruction's free-dim accumulator giving per-row partial sums (the
    memory-bound bulk of the problem -- ~287us at ~358 GB/s HBM/core).
  - The one-element-per-row label gather is done on device with
    gpsimd.indirect_dma_start over a flat [512*50257,1] view of the shard,
    indices = p*C + label computed on device (iota * C + labels, exact in f32
    since max index per 128-row block < 2^24; the 128-row block base goes in
    via element_offset).
  - numerator uses cos(arccos(t)+M) = t*cos(M) - sin(M)*sqrt(1-t^2).
  - log(D) for D ~ 1e30 exceeds the ScalarE Ln spline's valid range, so we
    compute ln(D * 2^-44) + 44*ln2 using the activation's free scale field.
  - Per-core output is the scalar sum of L over its 512 rows (reduced across
    partitions with a 1-column TensorE matmul). Host sums the 8 scalars and
    returns -total/B.
"""

import math
import sys

import numpy as np

if "/opt/trn_rl_repo" not in sys.path:
    sys.path.insert(0, "/opt/trn_rl_repo")

B, C = 4096, 50257
S_SCALE, MARGIN, EPS = 64.0, 0.5, 1e-7
N_CORES = 8
RPC = B // N_CORES  # 512 rows per core
P = 128  # SBUF partitions
NB = RPC // P  # 4 row blocks per core
NT = 8  # column tiles per row block
TC = -(-C // NT)  # 6283
LN_SHIFT = 44  # ln(D) = ln(D * 2^-44) + 44*ln2

_CACHE: dict = {}


def _build_nc():
    from concourse import bacc, bass, mybir
    import concourse.tile as tile

    f32 = mybir.dt.float32
    i32 = mybir.dt.int32
    Act = mybir.ActivationFunctionType
    Alu = mybir.AluOpType

    nc = bacc.Bacc("TRN2", target_bir_lowering=False, debug=False)

    wf1d = nc.dram_tensor("wf", [RPC * C], f32, kind="ExternalInput").ap()
    lab_t = nc.dram_tensor("labels_t", [P, NB], i32, kind="ExternalInput").ap()
    out_d = nc.dram_tensor("out", [1, 1], f32, kind="ExternalOutput").ap()

    wf2d = wf1d.rearrange("(r c) -> r c", c=C)  # [512, C]
    wf_flat = wf1d.rearrange("(n one) -> n one", one=1)  # [512*C, 1]

    cos_m, sin_m = math.cos(MARGIN), math.sin(MARGIN)

    with tile.TileContext(nc) as tc:
        with (
            tc.tile_pool(name="big", bufs=4) as big,
            tc.tile_pool(name="scratch", bufs=2) as scratch,
            tc.tile_pool(name="small", bufs=1) as small,
            tc.tile_pool(name="psum", bufs=1, space="PSUM") as psum,
        ):
            # ---- label gather: tvals[p, b] = wf[128*b + p, label[128*b + p]]
            # flat index = (128*b + p)*C + label; rowoff via iota
            # (channel_multiplier gives p*C, the free-dim pattern adds b*128*C),
            # then an exact int32 add on GPSIMD (DVE is fp32-internal and
            # 511*C+label exceeds 2^24).
            lab_i = small.tile([P, NB], i32)
            nc.sync.dma_start(out=lab_i[:], in_=lab_t)

            # Engine ALUs (DVE and the stock GPSIMD tensor ops) are fp32
            # internal, so any computed index must stay < 2^24 to be exact.
            # offs[p, b] = (p + 128*(b%2))*C + label  (max 255*C+50256 < 2^24);
            # the 2-block group base 2*128*C is an exact per-instruction
            # element_offset on two separate gathers. Iota pattern steps are
            # limited to int16, so generate raw p / b%2 and scale on DVE.
            iota_p = small.tile([P, 1], i32)
            nc.gpsimd.iota(iota_p[:], pattern=[[0, 1]], base=0, channel_multiplier=1)
            iota_b = small.tile([P, NB], i32)
            nc.gpsimd.iota(
                iota_b[:], pattern=[[0, 2], [1, 2]], base=0, channel_multiplier=0
            )
            p_f = small.tile([P, 1], f32)
            nc.vector.tensor_copy(out=p_f[:], in_=iota_p[:])
            b_f = small.tile([P, NB], f32)
            nc.vector.tensor_copy(out=b_f[:], in_=iota_b[:])
            lab_f = small.tile([P, NB], f32)
            nc.vector.tensor_copy(out=lab_f[:], in_=lab_i[:])

            row_f = small.tile([P, NB], f32)  # p + 128*(b%2)
            nc.vector.tensor_scalar(
                out=row_f[:], in0=b_f[:], scalar1=128.0, scalar2=p_f[:, :1],
                op0=Alu.mult, op1=Alu.add,
            )
            offm_f = small.tile([P, NB], f32)  # row * C
            nc.vector.tensor_scalar(
                out=offm_f[:], in0=row_f[:], scalar1=float(C), scalar2=None,
                op0=Alu.mult,
            )
            offs_f = small.tile([P, NB], f32)
            nc.vector.tensor_tensor(
                out=offs_f[:], in0=offm_f[:], in1=lab_f[:], op=Alu.add
            )
            offs_i = small.tile([P, NB], i32)
            nc.vector.tensor_copy(out=offs_i[:], in_=offs_f[:])

            # Compute-engine instructions only support one DMA-queue semaphore
            # wait each; funnel the two gather DMAs through two DVE copies so
            # every consumer of tvals waits on the DVE engine semaphore only.
            tv_raw = small.tile([P, NB], f32)
            tvals = small.tile([P, NB], f32)
            for g in range(2):
                nc.gpsimd.indirect_dma_start(
                    out=tv_raw[:, 2 * g : 2 * g + 2],
                    out_offset=None,
                    in_=wf_flat,
                    in_offset=bass.IndirectOffsetOnAxis(
                        ap=offs_i[:, 2 * g : 2 * g + 2], axis=0
                    ),
                    element_offset=g * 2 * P * C,
                )
                nc.vector.tensor_copy(
                    out=tvals[:, 2 * g : 2 * g + 2], in_=tv_raw[:, 2 * g : 2 * g + 2]
                )

            # ---- streaming exp-sum: partials[p, b*NT+t] = sum_c exp(S*tile)
            partials = small.tile([P, NB * NT], f32)
            for t in range(NT):
                c0 = t * TC
                w = min(TC, C - c0)
                for b in range(NB):
                    tin = big.tile([P, TC], f32, tag="in")
                    nc.sync.dma_start(
                        out=tin[:, :w], in_=wf2d[b * P : (b + 1) * P, c0 : c0 + w]
                    )
                    sc = scratch.tile([P, TC], f32, tag="sc")
                    k = b * NT + t
                    nc.scalar.activation(
                        out=sc[:, :w], in_=tin[:, :w], func=Act.Exp,
                        scale=S_SCALE, accum_out=partials[:, k : k + 1],
                    )

            rowsum = small.tile([P, NB], f32)
            for b in range(NB):
                nc.vector.tensor_reduce(
                    out=rowsum[:, b : b + 1],
                    in_=partials[:, b * NT : (b + 1) * NT],
                    axis=mybir.AxisListType.X,
                    op=Alu.add,
                )

            # ---- tail math on [P, NB] tiles
            exp_st = small.tile([P, NB], f32)  # exp(S * t), unclipped t
            nc.scalar.activation(out=exp_st[:], in_=tvals[:], func=Act.Exp, scale=S_SCALE)

            tcl = small.tile([P, NB], f32)  # clip(t)
            nc.vector.tensor_scalar(
                out=tcl[:], in0=tvals[:],
                scalar1=-1.0 + EPS, scalar2=1.0 - EPS,
                op0=Alu.max, op1=Alu.min,
            )
            tsq = small.tile([P, NB], f32)
            nc.scalar.activation(out=tsq[:], in_=tcl[:], func=Act.Square)
            omt = small.tile([P, NB], f32)  # 1 - t^2
            nc.scalar.activation(
                out=omt[:], in_=tsq[:], func=Act.Identity, bias=1.0, scale=-1.0
            )
            sq = small.tile([P, NB], f32)  # sqrt(1 - t^2)
            nc.scalar.activation(out=sq[:], in_=omt[:], func=Act.Sqrt)

            # numerator = S*cos_m * t - S*sin_m * sqrt(1-t^2)
            t1 = small.tile([P, NB], f32)
            nc.vector.tensor_scalar_mul(t1[:], sq[:], -S_SCALE * sin_m)
            t2 = small.tile([P, NB], f32)
            nc.vector.tensor_scalar_mul(t2[:], tcl[:], S_SCALE * cos_m)
            num = small.tile([P, NB], f32)
            nc.vector.tensor_tensor(out=num[:], in0=t2[:], in1=t1[:], op=Alu.add)

            expnum = small.tile([P, NB], f32)
            nc.scalar.activation(out=expnum[:], in_=num[:], func=Act.Exp)

            # D = exp(num) + rowsum - exp(S*t)
            excl = small.tile([P, NB], f32)
            nc.vector.tensor_tensor(
                out=excl[:], in0=rowsum[:], in1=exp_st[:], op=Alu.subtract
            )
            den = small.tile([P, NB], f32)
            nc.vector.tensor_tensor(
                out=den[:], in0=expnum[:], in1=excl[:], op=Alu.add
            )

            lnd = small.tile([P, NB], f32)  # ln(D) - LN_SHIFT*ln2
            nc.scalar.activation(
                out=lnd[:], in_=den[:], func=Act.Ln, scale=2.0 ** (-LN_SHIFT)
            )

            # L = num - lnd - LN_SHIFT*ln2
            lt = small.tile([P, NB], f32)
            nc.vector.tensor_tensor(
                out=lt[:], in0=num[:], in1=lnd[:], op=Alu.subtract
            )
            lfin = small.tile([P, NB], f32)
            nc.vector.tensor_scalar_add(lfin[:], lt[:], -LN_SHIFT * math.log(2.0))

            lrow = small.tile([P, 1], f32)
            nc.vector.tensor_reduce(
                out=lrow[:], in_=lfin[:], axis=mybir.AxisListType.X, op=Alu.add
            )

            ones = small.tile([P, 1], f32)
            nc.vector.memset(ones[:], 1.0)
            ps = psum.tile([1, 1], f32)
            nc.tensor.matmul(out=ps[:], lhsT=lrow[:], rhs=ones[:], start=True, stop=True)
            res = small.tile([1, 1], f32)
            nc.vector.tensor_copy(out=res[:], in_=ps[:])
            nc.sync.dma_start(out=out_d, in_=res[:])

    nc.compile()
    return nc


def get_nc():
    nc = _CACHE.get("nc")
    if nc is None:
        nc = _build_nc()
        _CACHE["nc"] = nc
    return nc


def make_in_maps(wf: np.ndarray, labels: np.ndarray):
    wf = np.asarray(wf, dtype=np.float32)
    labels = np.asarray(labels)
    in_maps = []
    for i in range(N_CORES):
        shard = np.ascontiguousarray(wf[i * RPC : (i + 1) * RPC]).reshape(-1)
        lab = labels[i * RPC : (i + 1) * RPC].astype(np.int32).reshape(NB, P).T
        in_maps.append({"wf": shard, "labels_t": np.ascontiguousarray(lab)})
    return in_maps


def kernel(wf: np.ndarray, labels: np.ndarray) -> np.ndarray:
    from concourse.bass_utils import run_bass_kernel_spmd

    nc = get_nc()
    in_maps = make_in_maps(wf, labels)
    out = run_bass_kernel_spmd(nc, in_maps, core_ids=list(range(N_CORES)))
    total = sum(float(r["out"][0, 0]) for r in out.results)
    return np.asarray(-total / B, dtype=np.float32)
